# revision 1
# baseline (speedup 1.0000x reference)
"""Trainium2 Bass kernel for LSTM-actor network (T=64, B=2048, OBS=48, H=256).

Strategy: data-parallel over batch B across 8 NeuronCores (256 envs/core).
Everything runs in a transposed ("feature-major") layout so the recurrent
matmul needs no per-step transposes:
  - state tiles are [128, 512] "pair layout": tile[p, k*256+b] = state[k*128+p, b]
  - gates computed as g.T = W.T @ [x;done;1;h*m] via PSUM accumulation
  - done-mask on c folded into the f-gate pre-activation (-30*done row)
  - sigmoid computed as 0.5*tanh(x/2)+0.5 so the whole kernel stays in the
    exp_and_others ACT table set (tanh+exp+square) -> zero table switches
  - LayerNorm stats via ones-matmul on PE; rsqrt via bit-trick+Newton on DVE
  - ELU(x)+1 = min(exp(x), relu(x)+1); the +1 shift folded into next bias
  - per-step MLP (LN -> 512 -> 256 -> heads) pipelined 8 steps behind scan
Output written feature-major [14, T*256] per core; host reassembles.
"""
import sys, os
sys.path.insert(0, "/opt/trn_rl_repo")
import numpy as np
import ml_dtypes
from contextlib import ExitStack

import concourse.bass as bass
import concourse.bacc as bacc
import concourse.tile as tile
from concourse import mybir
from concourse.bass_utils import run_bass_kernel_spmd

F32 = mybir.dt.float32
BF16 = mybir.dt.bfloat16
I32 = mybir.dt.int32
F32R = mybir.dt.float32r
AF = mybir.ActivationFunctionType
OP = mybir.AluOpType

T, B, OBS, H, M1, M2, A = 64, 2048, 48, 256, 512, 256, 12
NC_N = 8
BL = B // NC_N          # 256 envs per core
G4 = 4 * H              # 1024
LOG2PI = float(np.log(2.0 * np.pi))
LN_EPS = 1e-5
BIG = 30.0
C_LOGP = -(A / 2.0) * LOG2PI          # logp = -s + C_LOGP
C_ENT = A * (0.5 + 0.5 * LOG2PI)      # ent  =  s + C_ENT

RING = 8    # h ring depth
MLP_LAG = 6
ZCH = 16    # z0 staging chunk (steps)


def _bcast_ap(src):
    """DRAM row [1, n] -> partition-broadcast AP [[0,128],[...]]"""
    return bass.AP(tensor=src.tensor, offset=src.offset, ap=[[0, 128]] + src.ap[1:])


def _pair3(ap_2d):
    """[128, 512] -> [128, 2, 256] view"""
    return ap_2d.rearrange("p (k b) -> p k b", k=2)


def _pair3_bc(ap_2d):
    return ap_2d.rearrange("p (k b) -> p k b", k=2)


def _row3(ap_2d):
    """[128, 256] -> [128, 2(bcast), 256] 0-stride view"""
    return bass.AP(tensor=ap_2d.tensor, offset=ap_2d.offset,
                   ap=[ap_2d.ap[0], [0, 2], ap_2d.ap[1]])


def build_nc():
    nc = bacc.Bacc(None, target_bir_lowering=False)
    dt = nc.dram_tensor
    # per-core inputs
    z0_d = dt("z0", [64, T * BL], F32R, kind="ExternalInput")
    keep_d = dt("keep", [T, BL], F32, kind="ExternalInput")
    hm0_d = dt("hm0", [128, 2 * BL], F32R, kind="ExternalInput")
    c0_d = dt("c0p", [128, 2 * BL], F32, kind="ExternalInput")
    # replicated weights
    W0_d = dt("W0", [64, G4], F32R, kind="ExternalInput")
    Wh0_d = dt("Wh0", [128, G4], F32R, kind="ExternalInput")
    Wh1_d = dt("Wh1", [128, G4], F32R, kind="ExternalInput")
    W1_d = dt("W1", [H, M1], F32R, kind="ExternalInput")
    W2_d = dt("W2", [M1, M2], F32R, kind="ExternalInput")
    Whd_d = dt("Whd", [H, 128], F32R, kind="ExternalInput")
    b1_d = dt("b1r", [1, M1], F32R, kind="ExternalInput")
    b2_d = dt("b2r", [1, M2], F32R, kind="ExternalInput")
    bhd_d = dt("bhdr", [1, 128], F32R, kind="ExternalInput")
    onesmat_d = dt("onesmat", [128, 128], F32R, kind="ExternalInput")
    onesrow_d = dt("onesrow", [1, BL], F32R, kind="ExternalInput")
    # internal scratch
    stats_dram = dt("stats_scr", [T, 512], F32, kind="Internal")
    rk_dram = dt("rk_scr", [T, 512], F32, kind="Internal")
    s_dram = dt("s_scr", [T, BL], F32, kind="Internal")
    # output (feature-major)
    out_d = dt("out", [14, T * BL], F32, kind="ExternalOutput")

    with ExitStack() as ctx:
        tc = ctx.enter_context(tile.TileContext(nc))
        singles = ctx.enter_context(tc.tile_pool(name="singles", bufs=1))
        zpool = ctx.enter_context(tc.tile_pool(name="zpool", bufs=2))
        spool = ctx.enter_context(tc.tile_pool(name="spool", bufs=2))
        mpool = ctx.enter_context(tc.tile_pool(name="mpool", bufs=2))
        stpool = ctx.enter_context(tc.tile_pool(name="stpool", bufs=2))
        gps = ctx.enter_context(tc.tile_pool(name="gps", bufs=1, space="PSUM"))
        y1ps_p = ctx.enter_context(tc.tile_pool(name="y1ps", bufs=1, space="PSUM"))
        
        hdps_p = ctx.enter_context(tc.tile_pool(name="hdps", bufs=1, space="PSUM"))

        # ---- load weights & constants ----
        W0s = singles.tile([64, G4], F32R)
        nc.gpsimd.dma_start(out=W0s, in_=W0_d[:, :])
        Wh0s = singles.tile([128, G4], F32R)
        nc.gpsimd.dma_start(out=Wh0s, in_=Wh0_d[:, :])
        Wh1s = singles.tile([128, G4], F32R)
        nc.sync.dma_start(out=Wh1s, in_=Wh1_d[:, :])
        W1s = [singles.tile([128, M1], F32R, name=f"W1s{_k}") for _k in range(2)]
        for k in range(2):
            nc.sync.dma_start(out=W1s[k], in_=W1_d[k * 128:(k + 1) * 128, :])
        W2s = [singles.tile([128, M2], F32R, name=f"W2s{_k}") for _k in range(4)]
        for k in range(4):
            nc.gpsimd.dma_start(out=W2s[k], in_=W2_d[k * 128:(k + 1) * 128, :])
        Whds = [singles.tile([128, 128], F32R, name=f"Whds{_k}") for _k in range(2)]
        for k in range(2):
            nc.sync.dma_start(out=Whds[k], in_=Whd_d[k * 128:(k + 1) * 128, :])
        b1s = singles.tile([1, M1], F32R)
        nc.sync.dma_start(out=b1s, in_=b1_d[:, :])
        b2s = singles.tile([1, M2], F32R)
        nc.sync.dma_start(out=b2s, in_=b2_d[:, :])
        bhds = singles.tile([1, 128], F32R)
        nc.sync.dma_start(out=bhds, in_=bhd_d[:, :])
        onesmat = singles.tile([128, 128], F32R)
        nc.sync.dma_start(out=onesmat, in_=onesmat_d[:, :])
        ones12 = singles.tile([12, 1], BF16)
        nc.vector.memset(ones12, 1.0)
        onesrow = singles.tile([1, BL], F32R)
        nc.sync.dma_start(out=onesrow, in_=onesrow_d[:, :])
        c_state = singles.tile([128, 512], F32)
        nc.sync.dma_start(out=c_state, in_=c0_d[:, :])
        h_ring = [singles.tile([128, 512], F32R, name=f"hring{_k}") for _k in range(RING)]
        hm_cur = spool.tile([128, 512], F32R, tag="hm")
        nc.sync.dma_start(out=hm_cur, in_=hm0_d[:, :])

        zc_cur = zpool.tile([64, ZCH * BL], F32R, tag="zc")
        nc.sync.dma_start(out=zc_cur, in_=z0_d[:, 0:ZCH * BL])

        def ln_stats_math(u):
            st8 = stpool.tile([4, 512], F32, tag="st8")
            nc.sync.dma_start(out=st8, in_=stats_dram[u:u + 4, :])
            rk8 = stpool.tile([4, 512], F32, tag="rk8")
            mu = stpool.tile([4, 256], F32, tag="mu")
            tmp = stpool.tile([4, 256], F32, tag="tmp")
            v = stpool.tile([4, 256], F32, tag="vv")
            nc.gpsimd.tensor_scalar(mu, st8[:, 0:256], 1.0 / H, None, OP.mult)
            nc.gpsimd.tensor_scalar(v, st8[:, 256:512], 0.25 / H, LN_EPS, OP.mult, OP.add)
            nc.vector.tensor_tensor(tmp, mu, mu, OP.mult)
            nc.vector.scalar_tensor_tensor(v, tmp, -0.25, v, OP.mult, OP.add)
            y = rk8[:, 0:256]
            yi, vi = y.bitcast(I32), v.bitcast(I32)
            nc.vector.tensor_scalar(yi, vi, 1, None, OP.logical_shift_right)
            nc.vector.tensor_scalar(yi, yi, 0xFFFFFFFF, None, OP.bitwise_xor)
            nc.vector.tensor_scalar(yi, yi, 0x5F3759E0, None, OP.add)
            for it in range(3):
                nc.vector.tensor_tensor(tmp, y, y, OP.mult)
                nc.vector.tensor_tensor(tmp, tmp, v, OP.mult)
                if it < 2:
                    nc.vector.tensor_scalar(tmp, tmp, -0.5, 1.5, OP.mult, OP.add)
                else:   # fold rstd/2 into the last iteration
                    nc.vector.tensor_scalar(tmp, tmp, -0.25, 0.75, OP.mult, OP.add)
                nc.vector.tensor_tensor(y, y, tmp, OP.mult)
            nc.vector.tensor_tensor(rk8[:, 256:512], mu, y, OP.mult)
            nc.sync.dma_start(out=rk_dram[u:u + 4, :], in_=rk8)

        def g1_mms(u):
            """z + GEMM1 matmuls for step u (PE-early)."""
            rk_b = mpool.tile([128, 512], F32, tag="rkb")
            nc.sync.dma_start(out=rk_b[:, 0:256], in_=_bcast_ap(rk_dram[u:u + 1, 0:256]))
            nc.sync.dma_start(out=rk_b[:, 256:512], in_=_bcast_ap(rk_dram[u:u + 1, 256:512]))
            h = h_ring[u % RING]
            z = mpool.tile([128, 512], F32R, tag="z")
            nc.vector.tensor_tensor(_pair3(z), _pair3(h), _row3(rk_b[:, 0:256]), OP.mult)
            nc.vector.tensor_tensor(_pair3(z), _pair3(z), _row3(rk_b[:, 256:512]), OP.subtract)
            y1ps = y1ps_p.tile([128, 1024], F32, tag="y1")
            for m in range(4):
                o = y1ps[:, m * 256:(m + 1) * 256]
                nc.tensor.matmul(o, W1s[0][:, m * 128:(m + 1) * 128], z[:, 0:256], start=True, stop=False)
                nc.tensor.matmul(o, W1s[1][:, m * 128:(m + 1) * 128], z[:, 256:512], start=False, stop=False)
                nc.tensor.matmul(o, b1s[0:1, m * 128:(m + 1) * 128], onesrow, start=False, stop=True)
            return y1ps

        def elu1_tail(u, y1ps):
            # elu(v)+1 = max(min(exp(v), 1), v+1); v = y1ps (bias already in psum)
            e1x = mpool.tile([128, 1024], F32, tag="e1x")
            nc.scalar.activation(e1x, y1ps, AF.Exp)
            m1 = mpool.tile([128, 1024], F32, tag="m1")
            nc.gpsimd.tensor_scalar(m1, e1x, 1.0, None, OP.min)
            e1 = mpool.tile([128, 1024], F32R, tag="e1")
            nc.vector.scalar_tensor_tensor(e1, y1ps, 1.0, m1, OP.add, OP.max)
            return e1

        def g2_mms(u, e1):
            y2ps = y1ps_p.tile([128, 512], F32, tag="y2")
            for m in range(2):
                o = y2ps[:, m * 256:(m + 1) * 256]
                for k in range(4):
                    nc.tensor.matmul(o, W2s[k][:, m * 128:(m + 1) * 128], e1[:, k * 256:(k + 1) * 256], start=(k == 0), stop=False)
                nc.tensor.matmul(o, b2s[0:1, m * 128:(m + 1) * 128], onesrow, start=False, stop=True)
            return y2ps

        def elu2_tail(u, y2ps):
            e2x = mpool.tile([128, 512], F32, tag="e2x")
            nc.scalar.activation(e2x, y2ps, AF.Exp)
            m2 = mpool.tile([128, 512], F32, tag="m2")
            nc.gpsimd.tensor_scalar(m2, e2x, 1.0, None, OP.min)
            e2 = mpool.tile([128, 512], F32R, tag="e2")
            nc.vector.scalar_tensor_tensor(e2, y2ps, 1.0, m2, OP.add, OP.max)
            return e2

        def heads_mms(u, e2):
            hd = hdps_p.tile([128, 512], F32, tag="hd")
            o = hd[0:128, 0:256]
            nc.tensor.matmul(o, Whds[0][:, :], e2[:, 0:256], start=True, stop=False)
            nc.tensor.matmul(o, Whds[1][:, :], e2[:, 256:512], start=False, stop=False)
            nc.tensor.matmul(o, bhds[0:1, :], onesrow, start=False, stop=True)
            return hd

        def heads_mid(u, hd):
            ls_sb = mpool.tile([12, 256], BF16, tag="ls")
            nc.vector.tensor_scalar(ls_sb, hd[32:44, 0:256], -5.0, 2.0, OP.max, OP.min)
            nc.tensor.matmul(hd[32:33, 256:512], ones12, ls_sb, start=True, stop=True)

        def heads_tail(u, hd):
            hcopy = mpool.tile([44, 512], F32, tag="hcopy")
            nc.scalar.activation(hcopy[0:44, :], hd[0:44, 0:512], AF.Copy)
            nc.sync.dma_start(out=out_d[0:12, u * BL:(u + 1) * BL], in_=hcopy[0:12, 0:256])
            nc.sync.dma_start(out=s_dram[u:u + 1, :], in_=hcopy[32:33, 256:512])

        def stats_mms(t):
            h = h_ring[t % RING]
            hsq = hsq_tiles[t % 2]
            stp = hdps_p.tile([128, 512], F32, tag="hd")
            nc.tensor.matmul(stp[:, 0:256], onesmat, h[:, 0:256], start=True, stop=False)
            nc.tensor.matmul(stp[:, 0:256], onesmat, h[:, 256:512], start=False, stop=True)
            nc.tensor.matmul(stp[:, 256:512], onesmat, hsq[:, 0:256], start=True, stop=False)
            nc.tensor.matmul(stp[:, 256:512], onesmat, hsq[:, 256:512], start=False, stop=True)
            return stp

        def stats_qrow(t, stp):
            qrow = spool.tile([1, 512], F32, tag="qrow")
            nc.scalar.activation(qrow, stp[0:1, 0:512], AF.Copy)
            nc.sync.dma_start(out=stats_dram[t:t + 1, :], in_=qrow)

        hsq_tiles = [singles.tile([128, 512], F32R, name=f"hsqt{_k}") for _k in range(2)]
        e1_prev = None
        e2_prev = None

        def step(t, scan=True):
            nonlocal e1_prev, e2_prev, hm_cur, zc_cur, zc_next
            u1, u2, u3 = t - MLP_LAG, t - MLP_LAG - 1, t - MLP_LAG - 2
            if scan:
                if t % ZCH == ZCH // 2 and t + ZCH // 2 < T:
                    kchunk = (t + ZCH // 2) // ZCH
                    zc_next = zpool.tile([64, ZCH * BL], F32R, tag="zc")
                    nc.sync.dma_start(out=zc_next, in_=z0_d[:, kchunk * ZCH * BL:(kchunk + 1) * ZCH * BL])
                if t < T - 1:
                    mb = spool.tile([128, 256], F32, tag="mb")
                    nc.sync.dma_start(out=mb, in_=_bcast_ap(keep_d[t + 1:t + 2, :]))
                # scan gate matmuls first (recurrence-critical), f-gate block first
                g = gps.tile([128, 2048], F32, tag="g")
                zoff = (t % ZCH) * BL
                for gi in (1, 0, 2, 3):          # f, i, g, o
                    for mc in range(2):
                        m = gi * 2 + mc
                        o = g[:, gi * 512 + mc * 256: gi * 512 + (mc + 1) * 256]
                        nc.tensor.matmul(o, W0s[:, m * 128:(m + 1) * 128], zc_cur[:, zoff:zoff + BL], start=True, stop=False)
                        nc.tensor.matmul(o, Wh0s[:, m * 128:(m + 1) * 128], hm_cur[:, 0:256], start=False, stop=False)
                        nc.tensor.matmul(o, Wh1s[:, m * 128:(m + 1) * 128], hm_cur[:, 256:512], start=False, stop=True)
            stp = stats_mms(t - 1) if t >= 1 and t <= T else None
            # lagged MLP matmuls (inputs all computed in earlier steps)
            y1ps = g1_mms(u1) if 0 <= u1 < T else None
            y2ps = g2_mms(u2, e1_prev) if e1_prev is not None else None
            hd = heads_mms(u3, e2_prev) if e2_prev is not None else None
            if scan:
                # scan elementwise chain
                tif = spool.tile([128, 1024], F32, tag="tif")
                nc.scalar.activation(tif, g[:, 0:1024], AF.Tanh, scale=0.5)
                tng = spool.tile([128, 512], F32, tag="tng")
                nc.scalar.activation(tng, g[:, 1024:1536], AF.Tanh)
                tno = spool.tile([128, 512], F32, tag="tno")
                nc.scalar.activation(tno, g[:, 1536:2048], AF.Tanh, scale=0.5)
                a_t = spool.tile([128, 512], F32, tag="a")
                nc.vector.scalar_tensor_tensor(a_t, tif[:, 512:1024], 1.0, c_state, OP.add, OP.mult)
                p_t = spool.tile([128, 512], F32, tag="p")
                nc.vector.scalar_tensor_tensor(p_t, tif[:, 0:512], 1.0, tng, OP.add, OP.mult)
                csum = spool.tile([128, 512], F32, tag="csum")
                nc.vector.tensor_tensor(csum, a_t, p_t, OP.add)      # = 2*c_new
                if stp is not None:
                    stats_qrow(t - 1, stp)      # fills the ACT gap while DVE does csum
                if t < T - 1:
                    # som = (tanh_o+1)*keep/2 : off the critical chain (tno & mb ready early)
                    som = spool.tile([128, 512], F32, tag="som")
                    nc.vector.scalar_tensor_tensor(_pair3(som), _pair3_bc(tno), 1.0, _row3(mb), OP.add, OP.mult)
                tcn = spool.tile([128, 512], F32, tag="tc")
                nc.scalar.activation(tcn, csum, AF.Tanh, scale=0.5)  # tanh(c_new)
                if t < T - 1:
                    hm_next = spool.tile([128, 512], F32R, tag="hm")
                    nc.vector.tensor_tensor(hm_next, som, tcn, OP.mult)
                    hm_cur = hm_next
                h = h_ring[t % RING]                                  # stores h2 = 2h
                nc.vector.scalar_tensor_tensor(h, tno, 1.0, tcn, OP.add, OP.mult)
                nc.gpsimd.tensor_scalar(c_state, csum, 0.5, None, OP.mult)
            # late tails (full-step slack)
            if hd is not None:
                heads_mid(u3, hd)
            if not scan and stp is not None:
                stats_qrow(t - 1, stp)
            if t >= 5 and (t - 5) % 4 == 0 and t - 5 < T:
                ln_stats_math(t - 5)             # chunk stats fully stashed by t-1
            e1_new = elu1_tail(u1, y1ps) if y1ps is not None else None
            e2_prev = elu2_tail(u2, y2ps) if y2ps is not None else None
            e1_prev = e1_new
            if hd is not None:
                heads_tail(u3, hd)
            if scan:
                hsq = hsq_tiles[t % 2]
                nc.scalar.activation(hsq, h, AF.Square)
                if t % ZCH == ZCH - 1 and t < T - 1:
                    zc_cur = zc_next

        zc_next = None
        for t in range(T):
            step(t)
        for t in range(T, T + MLP_LAG + 3):
            step(t, scan=False)
        # logp / ent rows
        s_all = singles.tile([64, 256], F32)
        nc.sync.dma_start(out=s_all, in_=s_dram[:, :])
        lp = singles.tile([64, 256], F32)
        nc.vector.tensor_scalar(lp, s_all, -1.0, C_LOGP, OP.mult, OP.add)
        en = singles.tile([64, 256], F32)
        nc.vector.tensor_scalar(en, s_all, C_ENT, None, OP.add)
        nc.sync.dma_start(out=out_d[12:13, :].rearrange("o (t b) -> (o t) b", t=64), in_=lp)
        nc.sync.dma_start(out=out_d[13:14, :].rearrange("o (t b) -> (o t) b", t=64), in_=en)
    nc.finalize()
    return nc


_NC_CACHE = None


def kernel(x, h0, c0, W_ih, W_hh, b_ih, b_hh, ln_g, ln_b,
           W1, b1, W2, b2, Wm, bm, Ws, bs, done):
    global _NC_CACHE
    x = np.asarray(x, np.float32)
    done_f = np.asarray(done, np.float32)
    keep = 0.5 * (1.0 - done_f)   # mb_half: includes the 1/2 of sig(o)=(tanh+1)/2
    # ln affine folded into W1/b1: y = z*g + b -> W1' = g[:,None]*W1, b1' = b1 + b@W1
    W1f = (np.asarray(ln_g, np.float32)[:, None] * np.asarray(W1, np.float32))
    b1f = np.asarray(b1, np.float32) + np.asarray(ln_b, np.float32) @ np.asarray(W1, np.float32)
    W2f = np.asarray(W2, np.float32)
    b2f = np.asarray(b2, np.float32) - W2f.sum(axis=0)
    Whd = np.zeros((H, 128), np.float32)
    Whd[:, 0:12] = np.asarray(Wm, np.float32)
    Whd[:, 32:44] = np.asarray(Ws, np.float32)
    bhd = np.zeros((1, 128), np.float32)
    bhd[0, 0:12] = np.asarray(bm, np.float32) - np.asarray(Wm, np.float32).sum(axis=0)
    bhd[0, 32:44] = np.asarray(bs, np.float32) - np.asarray(Ws, np.float32).sum(axis=0)
    W0 = np.zeros((64, G4), np.float32)
    W0[0:OBS] = np.asarray(W_ih, np.float32).T
    W0[48, H:2 * H] = -BIG                      # f-gate done mask
    W0[49] = np.asarray(b_ih, np.float32) + np.asarray(b_hh, np.float32)
    WhT = np.asarray(W_hh, np.float32).T        # [256, 1024]

    shared = dict(W0=W0, Wh0=WhT[0:128].copy(), Wh1=WhT[128:256].copy(),
                  W1=W1f, W2=W2f, Whd=Whd,
                  b1r=b1f[None, :], b2r=b2f[None, :], bhdr=bhd,
                  onesmat=np.ones((128, 128), np.float32), onesrow=np.ones((1, BL), np.float32))

    def pair(mat):  # [BL, H] -> [128, 512] pair layout of mat.T
        mT = mat.T.astype(np.float32)            # [H, BL]
        return mT.reshape(2, 128, BL).transpose(1, 0, 2).reshape(128, 2 * BL).copy()

    in_maps = []
    for c in range(NC_N):
        sl = slice(c * BL, (c + 1) * BL)
        z0 = np.zeros((64, T, BL), np.float32)
        z0[0:OBS] = x[:, sl, :].transpose(2, 0, 1)
        z0[48] = done_f[:, sl]
        z0[49] = 1.0
        hm0 = pair(np.asarray(h0, np.float32)[sl] * (2.0 * keep[0, sl])[:, None])
        c0p = pair(np.asarray(c0, np.float32)[sl])
        m = dict(z0=z0.reshape(64, T * BL), keep=keep[:, sl].copy(),
                 hm0=hm0, c0p=c0p, **shared)
        in_maps.append(m)

    if _NC_CACHE is None:
        _NC_CACHE = build_nc()
    res = run_bass_kernel_spmd(_NC_CACHE, in_maps, core_ids=list(range(NC_N)))
    full = np.empty((T, B, 14), np.float32)
    for c in range(NC_N):
        oc = res.results[c]["out"].reshape(14, T, BL)
        full[:, c * BL:(c + 1) * BL, :] = oc.transpose(1, 2, 0)
    return full.reshape(T * B, 14)

